# revision 39
# baseline (speedup 1.0000x reference)
# Self-contained 8-core Trainium2 Bass kernel for the 2-layer GAT + mean-pool
# problem (nn_GAT_83820581749190).
#
# v3 (this session): ~2x over the v2 baseline (1638us -> ~820-960us).
#  - KEY FIX: L2 pad gather descriptors used to all point at ONE pad row,
#    hammering a single HBM bank; each DMA queue's drain collapsed to
#    ~35 desc/us and the kernel spent ~1.1ms in the L2 gather phase.
#    Pads now point at SCATTERED random rows (full bank parallelism,
#    ~350 desc/us) and are killed numerically instead: a host-built
#    padmask adds -1e9 into aldbc2 at pad (kk, block) positions, so the
#    pad slots' exp weights are 0 regardless of what data they gather.
#  - L2 gather calls use balanced equal-size chunks (<= 24 blocks each)
#    round-robin across the 4 SWDGE queues; gather pool 8-deep.
#  - global mean-pool matmuls hoisted out of the superblock loop (they
#    serialized agg(si+1) behind si's epilogue on the in-order PE queue).
#  - L1 psum->sbuf copies rebalanced 1:2 vector:scalar.
# v2 structure (kept): host-side edge-duplicated L1 (no gather), identity-
# matmul per-group aggregation with denominators riding the ones column,
# table2 AllGather between layers, per-block al_dst broadcast (aldbc).
import numpy as np
import ml_dtypes

N = 50000
E = 800000
IN = 128
HID = 32
HEADS = 4
OUT = 10
GPOOL = 64
NEG = 0.2
NCORES = 8
S = N // NCORES
LO_MAX = 32767          # max usable int16 gather index
SB_BLOCK_BUDGET = 32    # max gather blocks per superblock
SB_GROUP_BUDGET = 8     # max groups per superblock
GMAX = 32               # max blocks per dma_gather call (ring-friendly)
GMAX2 = 24              # max blocks per L2 dma_gather call
XCHUNK = 1024
SHARED_T2 = False

bf16 = ml_dtypes.bfloat16

# partition-major table1 layout: lo region = [128 partitions, W_LO words],
# hi region = [128, W_HI]; a "word" is one 256B row. Contiguous per-partition
# stores/loads (128 DMA descriptors instead of one per row); the gather index
# is just a host-side relabel j(row).
W_LO = 256
W_HI = 136


def _j_lo(r):
    return (r % 128) * W_LO + r // 128


def _j_hi(r):
    rp = r - 128 * W_LO
    return (rp % 128) * W_HI + rp // 128


def _ceil_to(v, m):
    return (v + m - 1) // m * m


def _balanced_chunks(n, gmax):
    """Split [0, n) into ceil(n/gmax) nearly-equal chunks."""
    if n <= 0:
        return []
    k = (n + gmax - 1) // gmax
    base, rem = divmod(n, k)
    out = []
    a = 0
    for i in range(k):
        b = a + base + (1 if i < rem else 0)
        out.append((a, b))
        a = b
    return out


# ======================= host prep =========================================

def _boundary_aware_order(deg_lo, deg_hi):
    """Sort ids by (lo desc, hi desc), but fill 128-groups that straddle a
    lo-run boundary from the *small-hi tail* of the next run, keeping
    per-group max_lo + max_hi tight."""
    Sn = len(deg_lo)
    base = np.lexsort((-deg_hi, -deg_lo))
    glo = deg_lo[base]
    runs = []
    i = 0
    while i < Sn:
        j = i
        while j < Sn and glo[j] == glo[i]:
            j += 1
        runs.append(list(base[i:j]))
        i = j
    order = []
    ri = 0
    fronts = [0] * len(runs)
    backs = [len(r) for r in runs]
    while len(order) < Sn:
        while ri < len(runs) and fronts[ri] >= backs[ri]:
            ri += 1
        if ri >= len(runs):
            break
        need = 128 - (len(order) % 128)
        avail = backs[ri] - fronts[ri]
        if avail >= need:
            order.extend(runs[ri][fronts[ri]:fronts[ri] + need])
            fronts[ri] += need
        else:
            order.extend(runs[ri][fronts[ri]:backs[ri]])
            fronts[ri] = backs[ri]
            need -= avail
            rj = ri + 1
            while need > 0 and rj < len(runs):
                a = backs[rj] - fronts[rj]
                t = min(a, need)
                order.extend(reversed(runs[rj][backs[rj] - t:backs[rj]]))
                backs[rj] -= t
                need -= t
                rj += 1
    P = np.asarray(order, np.int64)
    Ppos = np.empty(Sn, np.int64)
    Ppos[P] = np.arange(Sn)
    return P, Ppos


def _run_groups(glo, ghi, max_rows=128):
    Sn = len(glo)
    ng = (Sn + max_rows - 1) // max_rows
    dlo = np.zeros(ng, np.int64)
    dhi = np.zeros(ng, np.int64)
    for g in range(ng):
        s, e = g * max_rows, min((g + 1) * max_rows, Sn)
        dlo[g] = glo[s:e].max()
        dhi[g] = ghi[s:e].max()
    return dlo, dhi


def _build_layer(src, dstl, is_lo):
    deg_lo = np.bincount(dstl[is_lo], minlength=S)
    deg_hi = np.bincount(dstl[~is_lo], minlength=S)
    P, Ppos = _boundary_aware_order(deg_lo, deg_hi)
    dlo, dhi = _run_groups(deg_lo[P], deg_hi[P])
    return dict(src=src, dstl=dstl, is_lo=is_lo, deg_lo=deg_lo, deg_hi=deg_hi,
                P=P, Ppos=Ppos, dlo=dlo, dhi=dhi)


def _emit_slots(l, DLO, DHI, idx_lo_of, idx_hi_of, special_lo, special_hi):
    NG = len(DLO)
    src, is_lo = l["src"], l["is_lo"]
    Ppos = l["Ppos"]
    nreal = len(l["P"])
    slot2cmp = np.full(NG * 128, -1, np.int64)
    slot2cmp[:nreal] = np.arange(nreal)
    idx_lo = [np.full((int(DLO[g]), 128), special_lo, np.int64)
              for g in range(NG)]
    idx_hi = [np.full((int(DHI[g]), 128), special_hi, np.int64)
              for g in range(NG)]
    slot_of_edge = Ppos[l["dstl"]]
    order = np.argsort(slot_of_edge, kind="stable")
    for mask, arrs, idx_fn in ((is_lo, idx_lo, idx_lo_of),
                               (~is_lo, idx_hi, idx_hi_of)):
        m = mask[order]
        so = slot_of_edge[order][m]
        sr = src[order][m]
        jj = np.arange(len(so)) - np.searchsorted(so, so, side="left")
        gg, kk = so // 128, so % 128
        vals = idx_fn(sr)
        for g in range(NG):
            sel = gg == g
            if sel.any():
                arrs[g][jj[sel], kk[sel]] = vals[sel]
    return idx_lo, idx_hi, slot2cmp


def _wrap16(idx):
    """[n] -> [128, n//16] int16: idx i at [i%16, i//16], replicated x8."""
    n = len(idx)
    assert n % 16 == 0
    w = np.ascontiguousarray(np.asarray(idx).reshape(n // 16, 16).T)
    w = w.astype(np.int16)
    return np.tile(w, (8, 1))


def _wrap_groups(arrs):
    segs = [_wrap16(a.reshape(-1)) if a.size else np.zeros((128, 0), np.int16)
            for a in arrs]
    return np.concatenate(segs, axis=1) if segs else np.zeros((128, 0), np.int16)


def host_prep(x, edge_index, batch, W1, a1_src, a1_dst, b1, W2, a2_src, a2_dst,
              b2, Wl, bl):
    x = np.asarray(x, np.float32)
    edge_index = np.asarray(edge_index, np.int64)
    batch = np.asarray(batch, np.int64)
    # self-loops are handled in-kernel (per-group "self" block); the gather
    # only covers the real edges.
    src_all = edge_index[0]
    dst_all = edge_index[1]
    owner = dst_all // S

    a1_src = np.asarray(a1_src, np.float32)
    a1_dst = np.asarray(a1_dst, np.float32)
    W1 = np.asarray(W1, np.float32)
    W2 = np.asarray(W2, np.float32)
    As1 = np.zeros((HEADS * HID, HEADS), np.float32)
    Ad1 = np.zeros((HEADS * HID, HEADS), np.float32)
    for h in range(HEADS):
        As1[h * HID:(h + 1) * HID, h] = a1_src[h]
        Ad1[h * HID:(h + 1) * HID, h] = a1_dst[h]
    W1ext = np.concatenate([W1, W1 @ As1, W1 @ Ad1], axis=1)   # [128, 136]
    # W2ext': [zero | W2 | W2@a2s | W2@a2d] -> 35 cols; col 0 is the "ones"
    # denominator column (memset to 1 in scat_t, so table2 layout is
    # [ones | h2 | al2_src | al2_dst]).
    W2ext = np.concatenate(
        [np.zeros((HEADS * HID, 1), np.float32), W2,
         W2 @ np.asarray(a2_src, np.float32)[0][:, None],
         W2 @ np.asarray(a2_dst, np.float32)[0][:, None]], axis=1)  # [128,35]

    # pad-slot x row: x_pad @ (W1 @ As1) == -100 per head, so pad slots get
    # exp weight ~0 while contributing ~0 to the numerator.
    A = W1 @ As1                                    # [128, 4]
    x_pad = (A @ np.linalg.solve(A.T @ A + 1e-8 * np.eye(HEADS),
                                 -100.0 * np.ones(HEADS))).astype(np.float32)

    cores = [dict(c=c) for c in range(NCORES)]
    for cd in cores:
        c = cd["c"]
        m = owner == c
        cd["src"] = src_all[m]
        cd["dstl"] = dst_all[m] - c * S

    # ---------- layer 1 (host-side edge duplication, no device gather) ----
    for cd in cores:
        cd["l1"] = _build_layer(cd["src"], cd["dstl"],
                                np.ones(len(cd["src"]), bool))
    NG1 = max(len(cd["l1"]["dlo"]) for cd in cores)
    DLO1 = np.zeros(NG1, np.int64)
    for cd in cores:
        d = cd["l1"]
        DLO1[:len(d["dlo"])] = np.maximum(DLO1[:len(d["dlo"])], d["dlo"])
    DHI0 = np.zeros(NG1, np.int64)
    for cd in cores:
        # slot -> src node id (-1 for pad slots)
        s2s, _, cd["slot2cmp1"] = _emit_slots(
            cd["l1"], DLO1, DHI0, lambda s: s, lambda s: s, -1, -1)
        cd["slot2src"] = np.concatenate(
            [a.reshape(-1) for a in s2s]) if s2s else np.zeros(0, np.int64)

    # ---------- layer 2 ----------
    # core-5's own positions straddle LO_MAX; freeze its lo membership first
    cd5 = cores[5]
    alo = cd5["src"] // S <= 5
    da = np.bincount(cd5["dstl"][alo], minlength=S)
    db = np.bincount(cd5["dstl"][~alo], minlength=S)
    P5a, _ = _boundary_aware_order(da, db)
    n_lo5 = LO_MAX - 5 * (S + 1)
    lo5_set = np.zeros(S, bool)
    if n_lo5 > 0:
        lo5_set[P5a[:n_lo5]] = True

    def lo2_mask_of(src):
        ow = src // S
        lo = (ow <= 4).copy()
        m5 = ow == 5
        lo[m5] = lo5_set[src[m5] - 5 * S]
        return lo

    for cd in cores:
        cd["l2"] = _build_layer(cd["src"], cd["dstl"], lo2_mask_of(cd["src"]))
    l25 = cores[5]["l2"]
    idsA = np.where(lo5_set)[0]
    idsB = np.where(~lo5_set)[0]
    PA, _ = _boundary_aware_order(l25["deg_lo"][idsA], l25["deg_hi"][idsA])
    PB, _ = _boundary_aware_order(l25["deg_lo"][idsB], l25["deg_hi"][idsB])
    P5 = np.concatenate([idsA[PA], idsB[PB]])
    P5pos = np.empty(S, np.int64)
    P5pos[P5] = np.arange(S)
    l25["P"] = P5
    l25["Ppos"] = P5pos
    l25["dlo"], l25["dhi"] = _run_groups(l25["deg_lo"][P5], l25["deg_hi"][P5])

    # table2 layout: 8 AllGather segments of S+1 rows each, segment c =
    # [pad_row | core-c node rows]. Core 0's pad row is the lo pad (row 0),
    # core 7's is the hi pad (row 7*(S+1) > LO_MAX).
    row2_of = np.empty(N, np.int64)
    for cd in cores:
        c = cd["c"]
        row2_of[c * S:(c + 1) * S] = c * (S + 1) + 1 + cd["l2"]["Ppos"]
    NG2 = max(len(cd["l2"]["dlo"]) for cd in cores)
    DLO2 = np.zeros(NG2, np.int64)
    DHI2 = np.zeros(NG2, np.int64)
    for cd in cores:
        d = cd["l2"]
        DLO2[:len(d["dlo"])] = np.maximum(DLO2[:len(d["dlo"])], d["dlo"])
        DHI2[:len(d["dhi"])] = np.maximum(DHI2[:len(d["dhi"])], d["dhi"])
    for cd in cores:
        l2 = cd["l2"]
        assert (row2_of[l2["src"][l2["is_lo"]]] <= LO_MAX).all()
        assert (row2_of[l2["src"][~l2["is_lo"]]] > LO_MAX).all()
        cd["idx2_lo"], cd["idx2_hi"], cd["slot2cmp2"] = _emit_slots(
            l2, DLO2, DHI2,
            lambda s: row2_of[s], lambda s: row2_of[s] - (LO_MAX + 1),
            0, 7 * (S + 1) - (LO_MAX + 1))

    # ---------- aux ----------
    cnt = np.bincount(batch, minlength=GPOOL).astype(np.float32)
    recip_cnt = (1.0 / np.maximum(cnt, 1.0)).astype(np.float32)

    prep_DLO2, prep_DHI2 = DLO2, DHI2
    NG2h = len(DLO2)
    for cd in cores:
        c = cd["c"]
        gids = batch[c * S:(c + 1) * S]
        Mp = np.zeros((NG2 * 128, GPOOL), np.float32)
        s2c = cd["slot2cmp2"]
        real = s2c >= 0
        Mp[np.where(real)[0], gids[cd["l2"]["P"][s2c[real]]]] = 1.0
        # pre-transposed to [128, NG2*64] so the device load is contiguous
        cd["mpool"] = np.ascontiguousarray(
            Mp.reshape(NG2, 128, GPOOL).transpose(1, 0, 2)
            .reshape(128, NG2 * GPOOL)).astype(bf16)

        s2c1 = cd["slot2cmp1"]
        # h2sh rows: [pad | S nodes | trash]; dummy slots -> trash row S+1
        tgt = np.full(len(s2c1), S + 1, np.int64)
        r1 = s2c1 >= 0
        tgt[r1] = cd["l2"]["Ppos"][cd["l1"]["P"][s2c1[r1]]] + 1

        # own-node features, L1-position order, for the self/al table
        xtw = np.zeros((IN, 7 * XCHUNK), np.float32)
        xtw[:, cd["l1"]["Ppos"]] = x[c * S:(c + 1) * S].T
        cd["xTown"] = xtw.astype(bf16)
        # edge-duplicated features in slot order (the host-side "gather")
        s2src = cd["slot2src"]
        xd = np.where(s2src[:, None] >= 0, x[np.maximum(s2src, 0)],
                      x_pad[None, :])
        ncols = _ceil_to(max(len(s2src), 1), XCHUNK)
        xdt = np.zeros((IN, ncols), np.float32)
        xdt[:, :len(s2src)] = xd.T
        cd["xdupT"] = xdt.astype(bf16)

        # scatter pad reads across the table: identical pad indices hammer
        # one HBM bank and serialize the gather drain. Pads are killed via
        # the aldbc2 padmask instead, so any row works.
        rngp = np.random.RandomState(97 + c)
        spec_hi = 7 * (S + 1) - (LO_MAX + 1)
        for arrs, bound, special in ((cd["idx2_lo"], LO_MAX + 1, 0),
                                     (cd["idx2_hi"], 7 * (S + 1) -
                                      (LO_MAX + 1), spec_hi)):
            for a in arrs:
                m = a == special
                if m.any():
                    a[m] = rngp.randint(0, bound, int(m.sum()))
        # padmask in aldbc2 column layout: -1e9 at (kk, block) pad positions
        SB2m = _pack_superblocks(prep_DLO2, prep_DHI2, budget=48, ramp=True)
        lo_ofs = {}
        hi_ofs = {}
        ofs = 0
        for sbm in SB2m:
            for g in sbm:
                lo_ofs[g] = ofs
                ofs += int(prep_DLO2[g])
            for g in sbm:
                hi_ofs[g] = ofs
                ofs += int(prep_DHI2[g])
        pmask = np.zeros((128, max(ofs, 1)), np.float32)
        l2 = cd["l2"]
        dl_p = np.concatenate([l2["deg_lo"][l2["P"]],
                               np.zeros(NG2h * 128 - S, np.int64)])
        dh_p = np.concatenate([l2["deg_hi"][l2["P"]],
                               np.zeros(NG2h * 128 - S, np.int64)])
        for g in range(NG2h):
            dl = dl_p[g * 128:(g + 1) * 128]
            dh = dh_p[g * 128:(g + 1) * 128]
            for jj in range(int(prep_DLO2[g])):
                pmask[dl <= jj, lo_ofs[g] + jj] = -1e9
            for jj in range(int(prep_DHI2[g])):
                pmask[dh <= jj, hi_ofs[g] + jj] = -1e9
        cd["pmask"] = pmask

        cd["w_idx2lo"] = _wrap_groups(cd["idx2_lo"])
        cd["w_idx2hi"] = _wrap_groups(cd["idx2_hi"])
        cd["w_scat1"] = _wrap16(tgt)

    patch2 = np.zeros((1, 64), np.float32)
    patch2[:, 33] = -100.0   # al2_src of L2 pad rows; ones col (32) stays 0

    return dict(cores=cores,
                D1=[int(v) for v in DLO1],
                DLO2=[int(v) for v in DLO2], DHI2=[int(v) for v in DHI2],
                W1ext=W1ext.astype(bf16),
                W2ext=W2ext.astype(bf16),
                Wl=np.asarray(Wl, np.float32),
                b1=np.tile(np.asarray(b1, np.float32).reshape(1, -1),
                           (128, 1)),
                b2=np.tile(np.asarray(b2, np.float32).reshape(1, -1),
                           (128, 1)),
                bl=np.tile(np.asarray(bl, np.float32).reshape(1, -1),
                           (GPOOL, 1)),
                rcnt=np.tile(recip_cnt.reshape(1, -1), (HID, 1)),
                patch2=patch2,
                ident=np.eye(128, dtype=bf16))


def _pack_superblocks(DLO, DHI, budget=SB_BLOCK_BUDGET, gbudget=SB_GROUP_BUDGET,
                      ramp=False):
    sbs, cur, tot = [], [], 0

    def bud(i):
        if not ramp:
            return budget
        return (4, 4, 8, 8, 16, 16, 24, 24)[i] if i < 8 else budget

    for g in range(len(DLO)):
        d = int(DLO[g] + DHI[g])
        if cur and (tot + d > bud(len(sbs)) or len(cur) >= gbudget):
            sbs.append(cur)
            cur, tot = [], 0
        cur.append(g)
        tot += d
    if cur:
        sbs.append(cur)
    if ramp and len(sbs) > 2:
        # ramp-down: split the last two superblocks to shorten the tail
        tail, sbs = sbs[-2:], sbs[:-2]
        for sb in tail:
            if len(sb) >= 2:
                h = (len(sb) + 1) // 2
                sbs.append(sb[:h])
                sbs.append(sb[h:])
            else:
                sbs.append(sb)
    return sbs


def make_sched(prep):
    D1 = prep["D1"]
    DLO2, DHI2 = prep["DLO2"], prep["DHI2"]
    return dict(D1=D1, DLO2=DLO2, DHI2=DHI2,
                SB1=_pack_superblocks(D1, [0] * len(D1)),
                SB2=_pack_superblocks(DLO2, DHI2, budget=48, ramp=True),
                HASB1=bool(np.any(prep["b1"])), HASB2=bool(np.any(prep["b2"])),
                HASBL=bool(np.any(prep["bl"])))


# ======================= bass kernel =======================================

def build_bass(sc):
    import concourse.bacc as bacc
    import concourse.tile as tile
    import concourse.mybir as mybir
    from concourse.library_config import mlp

    dt = mybir.dt
    Alu = mybir.AluOpType
    Act = mybir.ActivationFunctionType
    Axis = mybir.AxisListType

    D1 = sc["D1"]
    DLO2, DHI2 = sc["DLO2"], sc["DHI2"]
    SB1, SB2 = sc["SB1"], sc["SB2"]
    HASB1 = sc.get("HASB1", True)
    HASB2 = sc.get("HASB2", True)
    HASBL = sc.get("HASBL", True)
    NG1, NG2 = len(D1), len(DLO2)
    TB1 = sum(D1)
    XD_COLS = _ceil_to(max(TB1 * 128, 1), XCHUNK)
    SH2_ROWS = NG1 * 128 + 128
    assert SH2_ROWS >= max(S + 2, NG2 * 128 + 1)
    T2ROWS = NCORES * (S + 1)

    nc = bacc.Bacc("TRN2", target_bir_lowering=False, debug=False,
                   num_devices=NCORES, num_swdge_queues=4,
                   dynamic_dma_scratch_size=32768)

    t_xTown = nc.dram_tensor("xTown", [IN, 7 * XCHUNK], dt.bfloat16,
                             kind="ExternalInput")
    t_xdup = nc.dram_tensor("xdupT", [IN, XD_COLS], dt.bfloat16,
                            kind="ExternalInput")
    t_w1 = nc.dram_tensor("w1ext", [IN, 136], dt.bfloat16,
                          kind="ExternalInput")
    t_w2 = nc.dram_tensor("w2ext", [IN, 35], dt.bfloat16, kind="ExternalInput")
    t_wl = nc.dram_tensor("wl", [HID, OUT], dt.float32, kind="ExternalInput")
    t_b1 = nc.dram_tensor("b1", [128, HEADS * HID], dt.float32,
                          kind="ExternalInput")
    t_b2 = nc.dram_tensor("b2", [128, HID], dt.float32, kind="ExternalInput")
    t_bl = nc.dram_tensor("bl", [GPOOL, OUT], dt.float32, kind="ExternalInput")
    t_rcnt = nc.dram_tensor("rcnt", [HID, GPOOL], dt.float32,
                            kind="ExternalInput")
    t_patch2 = nc.dram_tensor("patch2", [1, 64], dt.float32,
                              kind="ExternalInput")
    t_ident = nc.dram_tensor("ident", [128, 128], dt.bfloat16,
                             kind="ExternalInput")
    t_mpool = nc.dram_tensor("mpool", [128, NG2 * GPOOL], dt.bfloat16,
                             kind="ExternalInput")
    n2lo = max(8 * sum(DLO2), 8)
    n2hi = max(8 * sum(DHI2), 8)
    t_i2lo = nc.dram_tensor("idx2lo", [128, n2lo], dt.int16, kind="ExternalInput")
    t_i2hi = nc.dram_tensor("idx2hi", [128, n2hi], dt.int16, kind="ExternalInput")
    t_scat1 = nc.dram_tensor("scat1", [128, 8 * NG1], dt.int16,
                             kind="ExternalInput")
    TBL2E = max(sum(DLO2) + sum(DHI2), 1)
    t_pmask = nc.dram_tensor("padmask", [128, TBL2E], dt.float32,
                             kind="ExternalInput")
    t_out = nc.dram_tensor("out", [GPOOL, OUT], dt.float32,
                           kind="ExternalOutput")

    rg = [list(range(NCORES))]
    _qc = [0]

    def nextq():
        _qc[0] = (_qc[0] + 1) % 4
        return _qc[0]

    # superblock layout helpers (python-side)
    def sb_layout(SB, DLO, DHI):
        olo = np.concatenate([[0], np.cumsum(DLO)]).astype(int)
        ohi = np.concatenate([[0], np.cumsum(DHI)]).astype(int)
        # aldbc columns, superblock-major: [SB0 lo g0..gk | SB0 hi g0..gk | ..]
        lo_ofs = {}
        hi_ofs = {}
        sb_start = []
        ofs = 0
        for sb in SB:
            sb_start.append(ofs)
            for g in sb:
                lo_ofs[g] = ofs
                ofs += DLO[g]
            for g in sb:
                hi_ofs[g] = ofs
                ofs += DHI[g]
        total = ofs
        nbmax = max(sum(DLO[g] + DHI[g] for g in sb) for sb in SB)
        ngsb = max(len(sb) for sb in SB)
        return olo, ohi, lo_ofs, hi_ofs, sb_start, total, nbmax, ngsb

    with tile.TileContext(nc) as tc:
        with (
            tc.tile_pool(name="const", bufs=1) as constp,
            tc.tile_pool(name="dram", bufs=1, space="DRAM") as dramp,
        ):
            nc.gpsimd.load_library(mlp)

            t1own = dramp.tile([7 * XCHUNK, 128], dt.bfloat16, tag="t1own")
            t1own_v = t1own[:, :].rearrange("(p w) e -> p w e", w=56)
            table2 = dramp.tile([T2ROWS, 64], dt.float32,
                                tag="table2",
                                addr_space="Shared" if SHARED_T2 else "Local")
            h2sh = dramp.tile([SH2_ROWS, 64], dt.float32, tag="h2sh")
            cc_in = dramp.tile([HID, GPOOL], dt.float32, tag="ccin")
            cc_out = dramp.tile([HID, GPOOL], dt.float32, tag="ccout")

            w1_t = constp.tile([IN, 136], dt.bfloat16)
            nc.sync.dma_start(w1_t[:], t_w1[:])
            w2_t = constp.tile([IN, 35], dt.bfloat16)
            nc.sync.dma_start(w2_t[:], t_w2[:])
            wl_t = constp.tile([HID, OUT], dt.float32)
            nc.sync.dma_start(wl_t[:], t_wl[:])
            bl_t = constp.tile([GPOOL, OUT], dt.float32)
            nc.sync.dma_start(bl_t[:], t_bl[:])
            rc_t = constp.tile([HID, GPOOL], dt.float32)
            nc.sync.dma_start(rc_t[:], t_rcnt[:])
            id_t = constp.tile([128, 128], dt.bfloat16)
            nc.sync.dma_start(id_t[:], t_ident[:])
            if HASB1:
                b1_t = constp.tile([128, HEADS * HID], dt.float32)
                nc.sync.dma_start(b1_t[:], t_b1[:])
            if HASB2:
                b2_t = constp.tile([128, HID], dt.float32)
                nc.sync.dma_start(b2_t[:], t_b2[:])
            alds_sb = constp.tile([128, NG1, 8], dt.float32)
            mp_all = constp.tile([128, NG2, GPOOL], dt.bfloat16)
            nc.scalar.dma_start(
                mp_all[:], t_mpool[:, :].rearrange("p (g e) -> p g e", g=NG2))

            # zero the scatter_add target
            with tc.tile_pool(name="zp", bufs=1) as zp:
                z_t = zp.tile([128, SH2_ROWS // 128 * 64], dt.float32)
                nc.vector.memset(z_t[:], 0.0)
                nc.sync.dma_start(
                    h2sh[:, :].rearrange("(p k) e -> p (k e)", p=128), z_t[:])

            # ------------- mini phase X: own-node table + al values -------------
            with (
                tc.tile_pool(name="xload", bufs=3) as xlp,
                tc.tile_pool(name="xout", bufs=3) as xop,
                tc.tile_pool(name="xpsum", bufs=3, space="PSUM") as xpp,
            ):
                for t in range(7):
                    xt_t = xlp.tile([IN, XCHUNK], dt.bfloat16, tag="xt")
                    nc.scalar.dma_start(xt_t[:],
                                        t_xTown[:, t * XCHUNK:(t + 1) * XCHUNK])
                    o_t = xop.tile([128, 8, 128], dt.bfloat16, tag="xo")
                    for j0 in range(0, 8, 3):
                        j1 = min(j0 + 3, 8)
                        ps = xpp.tile([128, 3, 136], dt.float32, tag="xp")
                        for k in range(j0, j1):
                            nc.tensor.matmul(
                                ps[:, k - j0, :],
                                xt_t[:, k * 128:(k + 1) * 128],
                                w1_t[:], start=True, stop=True)
                        if (j0 // 3) % 2 == 0:
                            nc.vector.tensor_copy(
                                o_t[:, j0:j1, :], ps[:, 0:j1 - j0, 0:128])
                        else:
                            nc.scalar.activation(
                                o_t[:, j0:j1, :], ps[:, 0:j1 - j0, 0:128],
                                Act.Copy)
                        for k in range(j0, j1):
                            g = t * 8 + k
                            if g < NG1:
                                if g % 2 == 0:
                                    nc.vector.tensor_copy(
                                        alds_sb[:, g, :],
                                        ps[:, k - j0, 128:136])
                                else:
                                    nc.scalar.activation(
                                        alds_sb[:, g, :],
                                        ps[:, k - j0, 128:136], Act.Copy)
                    nc.sync.dma_start(t1own_v[:, 8 * t:8 * t + 8, :], o_t[:])
            with tc.tile_pool(name="patchp", bufs=1) as pp:
                p2_t = pp.tile([1, 64], dt.float32)
                nc.sync.dma_start(p2_t[:], t_patch2[:])
                nc.sync.dma_start(h2sh[0:1, :], p2_t[0:1, :])

            # ---------------- phase L1 (matmul-projected edge slots) --------
            o1 = np.concatenate([[0], np.cumsum(D1)]).astype(int)
            # aldbc columns are just the group-major block order
            NB1 = max(sum(D1[g] for g in sb) for sb in SB1)
            NGSB1 = max(len(sb) for sb in SB1)
            with (
                tc.tile_pool(name="l1pers", bufs=1) as persp,
                tc.tile_pool(name="idx1", bufs=1) as idxp,
                tc.tile_pool(name="xg1", bufs=4) as xgp,
                tc.tile_pool(name="gath1", bufs=4) as gathp,
                tc.tile_pool(name="small1", bufs=3) as smallp,
                tc.tile_pool(name="wh1", bufs=2) as whp,
                tc.tile_pool(name="hall1", bufs=2) as hallp,
                tc.tile_pool(name="epi1", bufs=2) as epip,
                tc.tile_pool(name="xps1", bufs=3, space="PSUM") as xpsp,
                tc.tile_pool(name="agg1", bufs=2, space="PSUM") as aggp,
                tc.tile_pool(name="psT1", bufs=1, space="PSUM") as psTp,
                tc.tile_pool(name="ps21", bufs=1, space="PSUM") as ps2p,
            ):
                # al_dst broadcast per block column (group-major order)
                aldbc = persp.tile([128, max(TB1, 1), 4], dt.float32,
                                   tag="aldbc")
                for g in range(NG1):
                    if D1[g] == 0:
                        continue
                    ad = alds_sb[:, g, 4:8]
                    nc.vector.scalar_tensor_tensor(
                        aldbc[:, o1[g]:o1[g] + D1[g], :],
                        ad.unsqueeze(1).broadcast_to((128, D1[g], 4)), 0.0,
                        ad.unsqueeze(1).broadcast_to((128, D1[g], 4)),
                        Alu.add, Alu.max)
                # self-block weights: exp(leaky(al_s + al_d)) per own node
                wself = persp.tile([128, NG1, 4], dt.bfloat16, tag="wself")
                wstmp = persp.tile([128, NG1, 4], dt.float32, tag="wstmp")
                nc.vector.tensor_tensor(wstmp[:], alds_sb[:, :, 0:4],
                                        alds_sb[:, :, 4:8], Alu.add)
                nc.vector.scalar_tensor_tensor(wstmp[:], wstmp[:], NEG,
                                               wstmp[:], Alu.mult, Alu.max)
                nc.scalar.activation(wself[:], wstmp[:], Act.Exp)
                scat_t = persp.tile([128, NG1, 64], dt.float32, tag="sc")
                nc.vector.memset(scat_t[:], 0.0)
                nc.vector.memset(scat_t[:, :, 0:1], 1.0)
                si_t = idxp.tile([128, 8 * NG1], dt.int16, tag="si")
                nc.sync.dma_start(si_t[:], t_scat1[:])

                for si, sb in enumerate(SB1):
                    g0 = sb[0]
                    ng = len(sb)
                    nb = sum(D1[g] for g in sb)
                    bofs = o1[g0]
                    # contiguous load of the edge-duplicated features
                    xg_t = xgp.tile([128, NB1 * 128], dt.bfloat16, tag="xg")
                    eng = nc.sync if si % 2 == 0 else nc.scalar
                    if nb > 0:
                        eng.dma_start(
                            xg_t[:, :nb * 128],
                            t_xdup[:, 128 * bofs:128 * (bofs + nb)])
                    # project: comb = [h(128) | al_src(4) | al_dst(4)]
                    comb_t = gathp.tile([128, NB1, 136], dt.bfloat16,
                                        tag="comb")
                    for j0 in range(0, nb, 3):
                        j1 = min(j0 + 3, nb)
                        ps = xpsp.tile([128, 3, 136], dt.float32, tag="xp")
                        for k in range(j0, j1):
                            nc.tensor.matmul(
                                ps[:, k - j0, :],
                                xg_t[:, 128 * k:128 * (k + 1)], w1_t[:],
                                start=True, stop=True)
                        if (j0 // 3) % 4 != 3:
                            nc.scalar.activation(
                                comb_t[:, j0:j1, :], ps[:, 0:j1 - j0, :],
                                Act.Copy)
                        else:
                            nc.vector.tensor_copy(
                                comb_t[:, j0:j1, :], ps[:, 0:j1 - j0, :])
                    aldsb_t = whp.tile([128, NGSB1, 128], dt.bfloat16,
                                       tag="aldsb")
                    nc.scalar.dma_start(aldsb_t[:, :ng, :],
                                        t1own_v[:, g0:g0 + ng, :])
                    # logits = al_src + al_dst, leaky, exp
                    logit_t = smallp.tile([128, NB1, 4], dt.float32, tag="lg")
                    if nb > 0:
                        nc.vector.scalar_tensor_tensor(
                            logit_t[:, :nb, :], comb_t[:, :nb, 128:132], 0.0,
                            aldbc[:, bofs:bofs + nb, :], Alu.add, Alu.add)
                        nc.vector.scalar_tensor_tensor(
                            logit_t[:, :nb, :], logit_t[:, :nb, :], NEG,
                            logit_t[:, :nb, :], Alu.mult, Alu.max)
                    w_t = smallp.tile([128, NB1, 4], dt.bfloat16, tag="wv")
                    if nb > 0:
                        nc.scalar.activation(w_t[:, :nb, :],
                                             logit_t[:, :nb, :], Act.Exp)
                        # weight the projected rows in place (h *= w per head)
                        nc.vector.tensor_tensor(
                            comb_t[:, :nb, 0:128].rearrange(
                                "p b (h c) -> p b h c", h=4),
                            comb_t[:, :nb, 0:128].rearrange(
                                "p b (h c) -> p b h c", h=4),
                            w_t[:, :nb, :].unsqueeze(3).broadcast_to(
                                (128, nb, 4, HID)), Alu.mult)
                    nc.vector.tensor_tensor(
                        aldsb_t[:, :ng, :].rearrange("p b (h c) -> p b h c",
                                                     h=4),
                        aldsb_t[:, :ng, :].rearrange("p b (h c) -> p b h c",
                                                     h=4),
                        wself[:, g0:g0 + ng, :].unsqueeze(3).broadcast_to(
                            (128, ng, 4, HID)), Alu.mult)
                    # denominators: per-group reduce over contiguous w blocks
                    den_t = smallp.tile([128, NGSB1, 4], dt.float32, tag="dn")
                    for gi, g in enumerate(sb):
                        if D1[g] > 0:
                            nc.vector.tensor_reduce(
                                den_t[:, gi, :],
                                w_t[:, o1[g] - bofs:o1[g] - bofs + D1[g], :]
                                .transpose([0, 2, 1]), axis=Axis.X,
                                op=Alu.add)
                    if any(D1[g] == 0 for g in sb):
                        for gi, g in enumerate(sb):
                            if D1[g] == 0:
                                nc.vector.memset(den_t[:, gi, :], 0.0)
                    nc.vector.tensor_tensor(den_t[:, :ng, :], den_t[:, :ng, :],
                                            wself[:, g0:g0 + ng, :], Alu.add)
                    rec_t = smallp.tile([128, NGSB1, 4], dt.float32, tag="rc")
                    nc.vector.reciprocal(rec_t[:, :ng, :], den_t[:, :ng, :])
                    # aggregate per group (identity matmul accumulate)
                    hall_t = hallp.tile([128, NGSB1, 128], dt.float32,
                                        tag="hall")
                    for gi, g in enumerate(sb):
                        agg = aggp.tile([128, 128], dt.float32, tag="agg")
                        rhss = ([comb_t[:, o1[g] - bofs + b, 0:128]
                                 for b in range(D1[g])]
                                + [aldsb_t[:, gi, :]])
                        for bi, rhs in enumerate(rhss):
                            nc.tensor.matmul(agg[:], id_t[:], rhs,
                                             start=(bi == 0),
                                             stop=(bi == len(rhss) - 1))
                        nc.scalar.activation(hall_t[:, gi, :], agg[:],
                                             Act.Copy)
                    # epilogue for the whole superblock
                    scaled_t = epip.tile([128, NGSB1, 128], dt.float32,
                                         tag="sd")
                    nc.vector.tensor_tensor(
                        scaled_t[:, :ng, :].rearrange("p b (h c) -> p b h c",
                                                      h=4),
                        hall_t[:, :ng, :].rearrange("p b (h c) -> p b h c",
                                                    h=4),
                        rec_t[:, :ng, :].unsqueeze(3).broadcast_to(
                            (128, ng, 4, HID)), Alu.mult)
                    if HASB1:
                        nc.vector.tensor_tensor(
                            scaled_t[:, :ng, :], scaled_t[:, :ng, :],
                            b1_t[:].unsqueeze(1).broadcast_to((128, ng, 128)),
                            Alu.add)
                    tmp_t = epip.tile([128, NGSB1, 128], dt.float32, tag="tm")
                    nc.scalar.activation(tmp_t[:, :ng, :], scaled_t[:, :ng, :],
                                         Act.Relu, scale=-1.0)
                    nc.scalar.activation(tmp_t[:, :ng, :], tmp_t[:, :ng, :],
                                         Act.Exp, scale=-1.0)
                    elusb_t = epip.tile([128, NGSB1, 128], dt.bfloat16,
                                        tag="elu")
                    nc.vector.scalar_tensor_tensor(
                        elusb_t[:, :ng, :], tmp_t[:, :ng, :], -1.0,
                        scaled_t[:, :ng, :], Alu.add, Alu.max)
                    # h2 projection per group
                    for gi, g in enumerate(sb):
                        psT = psTp.tile([128, 128], dt.bfloat16, tag="pt")
                        nc.tensor.transpose(psT[:], elusb_t[:, gi, :], id_t[:])
                        eluT_t = epip.tile([128, 128], dt.bfloat16, tag="et")
                        nc.scalar.activation(eluT_t[:], psT[:], Act.Copy)
                        ps2 = ps2p.tile([128, 35], dt.float32, tag="p2")
                        nc.tensor.matmul(ps2[:], eluT_t[:], w2_t[:],
                                         start=True, stop=True)
                        nc.scalar.activation(scat_t[:, g, 1:35],
                                             ps2[:, 1:35], Act.Copy)
                    nc.gpsimd.dma_scatter_add(
                        h2sh[0:S + 2, :], scat_t[:, g0:g0 + ng, :],
                        si_t[:, 8 * g0:8 * (g0 + ng)],
                        128 * ng, 128 * ng, 64,
                        single_packet=False, queue_num=nextq())

            # ---------------- exchange ----------------
            nc.gpsimd.collective_compute(
                "AllGather", mybir.AluOpType.bypass, replica_groups=rg,
                ins=[h2sh[0:S + 1, :]], outs=[table2[0:T2ROWS, :]])

            # ---------------- phase L2 ----------------
            tab2_lo = table2[0:LO_MAX + 1, :]
            tab2_hi = table2[LO_MAX + 1:T2ROWS, :]
            olo2, ohi2, lo_ofs2, hi_ofs2, sbst2, TBL2, NB2, NGSB2 = \
                sb_layout(SB2, DLO2, DHI2)
            with (
                tc.tile_pool(name="l2pers", bufs=1) as persp,
                tc.tile_pool(name="idx2", bufs=4) as idxp,
                tc.tile_pool(name="gath2", bufs=8) as gathp,
                tc.tile_pool(name="small2", bufs=3) as smallp,
                tc.tile_pool(name="wh2", bufs=2) as whp,
                tc.tile_pool(name="hall2", bufs=2) as hallp,
                tc.tile_pool(name="epi2", bufs=2) as epip,
                tc.tile_pool(name="agg2", bufs=2, space="PSUM") as aggp,
                tc.tile_pool(name="poolps", bufs=1, space="PSUM") as poolpp,
            ):
                # own rows [h2|ones|al2s|al2d] in L2 group-slot layout.
                # Strided load split across both HWDGE engines; depends only
                # on the scatter, so it overlaps the AllGather.
                ald2_all = persp.tile([128, NG2, 64], dt.float32, tag="ald2")
                qs = [0, NG2 // 4, NG2 // 2, 3 * NG2 // 4, NG2]
                for qi in range(4):
                    a, b = qs[qi], qs[qi + 1]
                    eng = nc.sync if qi % 2 == 0 else nc.scalar
                    eng.dma_start(
                        ald2_all[:, a:b, :],
                        h2sh[1 + 128 * a:1 + 128 * b, :].rearrange(
                            "(b p) e -> p b e", p=128))
                # preload whole idx arrays
                i2lo_all = persp.tile([128, n2lo], dt.int16, tag="i2lo")
                nc.sync.dma_start(i2lo_all[:], t_i2lo[:])
                i2hi_all = persp.tile([128, n2hi], dt.int16, tag="i2hi")
                nc.scalar.dma_start(i2hi_all[:], t_i2hi[:])
                aldbc2 = persp.tile([128, max(TBL2, 1), 1], dt.float32,
                                    tag="aldbc2")
                pm_t = persp.tile([128, TBL2E, 1], dt.float32, tag="pmask")
                nc.sync.dma_start(
                    pm_t[:], t_pmask[:, :].rearrange("p (b e) -> p b e", e=1))
                for g in range(NG2):
                    ad = ald2_all[:, g, 34:35]
                    if DLO2[g] > 0:
                        nc.vector.scalar_tensor_tensor(
                            aldbc2[:, lo_ofs2[g]:lo_ofs2[g] + DLO2[g], :],
                            ad.unsqueeze(1).broadcast_to((128, DLO2[g], 1)),
                            0.0,
                            ad.unsqueeze(1).broadcast_to((128, DLO2[g], 1)),
                            Alu.add, Alu.max)
                    if DHI2[g] > 0:
                        nc.vector.scalar_tensor_tensor(
                            aldbc2[:, hi_ofs2[g]:hi_ofs2[g] + DHI2[g], :],
                            ad.unsqueeze(1).broadcast_to((128, DHI2[g], 1)),
                            0.0,
                            ad.unsqueeze(1).broadcast_to((128, DHI2[g], 1)),
                            Alu.add, Alu.max)
                # kill pad slots: logits of pad (kk, block) positions get
                # -1e9 so their exp weight is 0 regardless of gathered data.
                nc.vector.tensor_tensor(aldbc2[:, 0:TBL2E, :],
                                        aldbc2[:, 0:TBL2E, :], pm_t[:],
                                        Alu.add)
                poolps = poolpp.tile([HID, GPOOL], dt.float32)
                wself2 = persp.tile([128, NG2, 1], dt.float32, tag="wself2")
                nc.vector.tensor_tensor(wself2[:], ald2_all[:, :, 33:34],
                                        ald2_all[:, :, 34:35], Alu.add)
                nc.vector.scalar_tensor_tensor(wself2[:], wself2[:], NEG,
                                               wself2[:], Alu.mult, Alu.max)
                nc.scalar.activation(wself2[:], wself2[:], Act.Exp)
                h2p_all = persp.tile([128, NG2, HID], dt.bfloat16, tag="h2p")

                for si, sb in enumerate(SB2):
                    g0 = sb[0]
                    ng = len(sb)
                    nlo = sum(DLO2[g] for g in sb)
                    nhi = sum(DHI2[g] for g in sb)
                    nb = nlo + nhi
                    comb_t = gathp.tile([128, NB2, 64], dt.float32, tag="comb")
                    for a, b in _balanced_chunks(nlo, GMAX2):
                        nc.gpsimd.dma_gather(
                            comb_t[:, a:b, :], tab2_lo,
                            i2lo_all[:, 8 * (olo2[g0] + a):
                                     8 * (olo2[g0] + b)],
                            128 * (b - a), 128 * (b - a), 64,
                            single_packet=False, queue_num=nextq())
                    for a, b in _balanced_chunks(nhi, GMAX2):
                        nc.gpsimd.dma_gather(
                            comb_t[:, nlo + a:nlo + b, :], tab2_hi,
                            i2hi_all[:, 8 * (ohi2[g0] + a):
                                     8 * (ohi2[g0] + b)],
                            128 * (b - a), 128 * (b - a), 64,
                            single_packet=False, queue_num=nextq())
                    logit_t = smallp.tile([128, NB2, 1], dt.float32, tag="lg")
                    nc.vector.scalar_tensor_tensor(
                        logit_t[:, :nb, :], comb_t[:, :nb, 33:34], 0.0,
                        aldbc2[:, sbst2[si]:sbst2[si] + nb, :], Alu.add,
                        Alu.add)
                    nc.vector.scalar_tensor_tensor(
                        logit_t[:, :nb, :], logit_t[:, :nb, :], NEG,
                        logit_t[:, :nb, :], Alu.mult, Alu.max)
                    w_t = smallp.tile([128, NB2, 1], dt.float32, tag="wv")
                    nc.scalar.activation(w_t[:, :nb, :], logit_t[:, :nb, :],
                                         Act.Exp)
                    # [w*h | w] via the ones column (col 32 of table2 rows)
                    wh_t = whp.tile([128, NB2, 33], dt.bfloat16, tag="wh")
                    nc.vector.tensor_tensor(
                        wh_t[:, :nb, :], comb_t[:, :nb, 0:33],
                        w_t[:, :nb, :].broadcast_to((128, nb, 33)), Alu.mult)
                    whs_t = whp.tile([128, NGSB2, 33], dt.bfloat16, tag="whs")
                    nc.vector.tensor_tensor(
                        whs_t[:, :ng, 1:33], ald2_all[:, g0:g0 + ng, 1:33],
                        wself2[:, g0:g0 + ng, :].broadcast_to((128, ng, 32)),
                        Alu.mult)
                    nc.vector.tensor_copy(whs_t[:, :ng, 0:1],
                                          wself2[:, g0:g0 + ng, :])
                    hall_t = hallp.tile([128, NGSB2, 33], dt.float32,
                                        tag="hall")
                    lo_off = 0
                    hi_off = 0
                    for gi, g in enumerate(sb):
                        dlo, dhi = DLO2[g], DHI2[g]
                        agg = aggp.tile([128, 33], dt.float32, tag="agg")
                        rhss = ([wh_t[:, lo_off + b, :] for b in range(dlo)]
                                + [wh_t[:, nlo + hi_off + b, :]
                                   for b in range(dhi)]
                                + [whs_t[:, gi, :]])
                        for bi, rhs in enumerate(rhss):
                            nc.tensor.matmul(agg[:], id_t[:], rhs,
                                             start=(bi == 0),
                                             stop=(bi == len(rhss) - 1))
                        nc.scalar.activation(hall_t[:, gi, :], agg[:],
                                             Act.Copy)
                        lo_off += dlo
                        hi_off += dhi
                    rec_t = smallp.tile([128, NGSB2, 1], dt.float32, tag="rc")
                    nc.vector.reciprocal(rec_t[:, :ng, :],
                                         hall_t[:, :ng, 0:1])
                    scaled_t = epip.tile([128, NGSB2, HID], dt.float32,
                                         tag="sd")
                    nc.vector.tensor_tensor(
                        scaled_t[:, :ng, :], hall_t[:, :ng, 1:33],
                        rec_t[:, :ng, :].broadcast_to((128, ng, HID)),
                        Alu.mult)
                    if HASB2:
                        nc.vector.tensor_tensor(
                            scaled_t[:, :ng, :], scaled_t[:, :ng, :],
                            b2_t[:].unsqueeze(1).broadcast_to((128, ng, HID)),
                            Alu.add)
                    tmp_t = epip.tile([128, NGSB2, HID], dt.float32, tag="tm")
                    nc.scalar.activation(tmp_t[:, :ng, :], scaled_t[:, :ng, :],
                                         Act.Relu, scale=-1.0)
                    nc.scalar.activation(tmp_t[:, :ng, :], tmp_t[:, :ng, :],
                                         Act.Exp, scale=-1.0)
                    nc.vector.scalar_tensor_tensor(
                        h2p_all[:, g0:g0 + ng, :], tmp_t[:, :ng, :], -1.0,
                        scaled_t[:, :ng, :], Alu.add, Alu.max)

                # pool matmuls hoisted out of the superblock loop: keeping
                # them inline serialized agg(si+1) behind the full epilogue
                # of si on the in-order Tensor queue.
                for g in range(NG2):
                    nc.tensor.matmul(poolps[:], h2p_all[:, g, :],
                                     mp_all[:, g, :],
                                     start=(g == 0), stop=(g == NG2 - 1))

                # ------------- pool + final linear -------------
                with tc.tile_pool(name="fin", bufs=1) as finp, \
                        tc.tile_pool(name="finps", bufs=1, space="PSUM") as fpp:
                    poolsb = finp.tile([HID, GPOOL], dt.float32)
                    nc.vector.tensor_copy(poolsb[:], poolps[:])
                    nc.sync.dma_start(cc_in[:, :], poolsb[:])
                    nc.gpsimd.collective_compute(
                        "AllReduce", Alu.add, replica_groups=rg,
                        ins=[cc_in[:, :]], outs=[cc_out[:, :]])
                    psum_t = finp.tile([HID, GPOOL], dt.float32)
                    nc.sync.dma_start(psum_t[:], cc_out[:, :])
                    mean_t = finp.tile([HID, GPOOL], dt.float32)
                    nc.vector.tensor_tensor(mean_t[:], psum_t[:], rc_t[:],
                                            Alu.mult)
                    psO = fpp.tile([GPOOL, OUT], dt.float32)
                    nc.tensor.matmul(psO[:], mean_t[:], wl_t[:], start=True,
                                     stop=True)
                    out_t = finp.tile([GPOOL, OUT], dt.float32)
                    if HASBL:
                        nc.vector.tensor_tensor(out_t[:], psO[:], bl_t[:],
                                                Alu.add)
                    else:
                        nc.vector.tensor_copy(out_t[:], psO[:])
                    nc.sync.dma_start(t_out[:, :], out_t[:])

    nc.compile()
    return nc


def core_inputs(prep, c):
    cd = prep["cores"][c]

    def padcols(a, cols):
        if a.shape[1] == cols:
            return a
        out = np.zeros((a.shape[0], cols), a.dtype)
        out[:, :a.shape[1]] = a
        return out

    n2lo = max(8 * sum(prep["DLO2"]), 8)
    n2hi = max(8 * sum(prep["DHI2"]), 8)
    xd_cols = _ceil_to(max(sum(prep["D1"]) * 128, 1), XCHUNK)
    xd = cd["xdupT"]
    if xd.shape[1] != xd_cols:
        t = np.zeros((IN, xd_cols), xd.dtype)
        t[:, :xd.shape[1]] = xd
        xd = t
    return dict(
        xTown=np.ascontiguousarray(cd["xTown"]),
        xdupT=np.ascontiguousarray(xd),
        w1ext=prep["W1ext"], w2ext=prep["W2ext"],
        wl=prep["Wl"],
        b1=prep["b1"], b2=prep["b2"], bl=prep["bl"], rcnt=prep["rcnt"],
        patch2=prep["patch2"], ident=prep["ident"],
        mpool=np.ascontiguousarray(cd["mpool"]),
        idx2lo=padcols(cd["w_idx2lo"], n2lo),
        idx2hi=padcols(cd["w_idx2hi"], n2hi),
        scat1=cd["w_scat1"],
        padmask=np.ascontiguousarray(cd["pmask"]),
    )


_CACHE = {}


def kernel(**inputs):
    from concourse.bass_utils import run_bass_kernel_spmd

    inputs = {k: np.asarray(v) for k, v in inputs.items()}
    prep = host_prep(**inputs)
    sc = make_sched(prep)
    key = str(sc)
    if key not in _CACHE:
        _CACHE[key] = build_bass(sc)
    nc = _CACHE[key]
    in_maps = [core_inputs(prep, c) for c in range(NCORES)]
    res = run_bass_kernel_spmd(nc, in_maps, list(range(NCORES)))
    return np.asarray(res.results[0]["out"], np.float32)

